# revision 1
# baseline (speedup 1.0000x reference)
"""Gaussian smoother: out[b,n] = sum_t x[b,t,n] * w[t] on 8 trn2 cores.

Full input x:[64,2048,1024] f32 -> out:[64,1024] f32.
Data-parallel over batch: core i handles x[i*8:(i+1)*8].
Per core: t-chunks of 128 go on SBUF partitions, PE matmul contracts them
against the Gaussian weight column (lhsT=[128,1]), accumulating the 16
chunks of T=2048 into one PSUM bank per (batch, n-half) group.

The weight vector is pre-arranged on the host to [128, 16] (w2d[p,c] =
w[c*128+p]) so its DMA is a plain contiguous 128-partition transfer —
the strided 4-byte-element cast DMA of a flat [2048] w crashes the
device (NRT_EXEC_UNIT_UNRECOVERABLE).
"""

import numpy as np

SIGMA = 20.0
B_FULL, T, N = 64, 2048, 1024
N_CORES = 8
B_LOC = B_FULL // N_CORES  # 8
P = 128
C = T // P  # 16 t-chunks
NF = 512  # matmul moving free dim (one PSUM bank of f32)
NH = N // NF  # 2 n-halves

# matmul input dtype: "f32r" (1 cyc/row, tf32-ish), "bf16" (cast in DMA), "f32" (4 cyc/row)
MM_DTYPE = "bf16"
X_BUFS = 2
DMA_ONLY = False  # timing diagnostics: skip matmuls/copies, keep the x DMAs
# "swdge": gpsimd cast-DMA (f32->bf16 inline, ~327 GB/s);
# "dve": sync HWDGE f32 DMA (~350 GB/s) + DVE cast to bf16 on-chip.
DMA_CAST = "dve"

_compiled = None


def _gauss_weights() -> np.ndarray:
    x = np.arange(T, dtype=np.float64)
    k = np.exp(-0.5 * ((x - T // 2) / SIGMA) ** 2)
    k = k / k.sum()
    return k.astype(np.float32)


def _emit(tc, out, x, w, repeats: int = 1):
    import concourse.mybir as mybir

    nc = tc.nc
    f32 = mybir.dt.float32
    if MM_DTYPE == "bf16":
        sb_dt = mybir.dt.bfloat16
    elif MM_DTYPE == "f32r":
        sb_dt = mybir.dt.float32r
    else:
        sb_dt = f32

    with (
        tc.tile_pool(name="wp", bufs=1) as wpool,
        tc.tile_pool(name="xp", bufs=X_BUFS) as xpool,
        tc.tile_pool(name="ps", bufs=4, space="PSUM") as pspool,
        tc.tile_pool(name="op", bufs=1) as opool,
    ):
        # w arrives host-prepared as [P, C] f32; contiguous DMA, cast on-chip.
        w_f32 = wpool.tile([P, C], f32)
        nc.sync.dma_start(out=w_f32[:], in_=w)
        if sb_dt != f32:
            w_sb = wpool.tile([P, C], sb_dt)
            nc.vector.tensor_copy(out=w_sb[:], in_=w_f32[:])
        else:
            w_sb = w_f32

        use_dve_cast = sb_dt != f32 and DMA_CAST == "dve"
        CH = C // 2 if use_dve_cast else C  # t-chunks per x tile

        def load_batch(b):
            """DMA (and maybe cast) batch b; returns list of chunk tiles."""
            if not use_dve_cast:
                xt = xpool.tile([P, C, N], sb_dt, tag="xt")
                x_dma = nc.gpsimd if sb_dt != f32 else nc.sync
                x_dma.dma_start(out=xt[:], in_=x[b].rearrange("(c p) n -> p c n", p=P))
                return [xt]
            halves = []
            for h in range(2):
                xf = xpool.tile([P, CH, N], f32, tag="xf")
                nc.sync.dma_start(
                    out=xf[:],
                    in_=x[b, h * (T // 2) : (h + 1) * (T // 2)].rearrange(
                        "(c p) n -> p c n", p=P
                    ),
                )
                xb = xpool.tile([P, CH, N], sb_dt, tag="xb")
                nc.vector.tensor_copy(out=xb[:], in_=xf[:])
                halves.append(xb)
            return halves

        def one_pass():
            out_sb = None if DMA_ONLY else opool.tile([1, B_LOC * N], f32)
            for b in range(B_LOC):
                tiles = load_batch(b)
                if DMA_ONLY:
                    continue
                for nh in range(NH):
                    ps = pspool.tile([1, NF], f32)
                    for c in range(C):
                        xt = tiles[c // CH]
                        nc.tensor.matmul(
                            ps[:],
                            lhsT=w_sb[:, c : c + 1],
                            rhs=xt[:, c % CH, nh * NF : (nh + 1) * NF],
                            start=(c == 0),
                            stop=(c == C - 1),
                        )
                    nc.scalar.copy(
                        out=out_sb[:, b * N + nh * NF : b * N + (nh + 1) * NF],
                        in_=ps[:],
                    )
            if DMA_ONLY:
                return
            # NB: keep both sides of the DMA 2-D ([1, B*N]) — a flat 1-D AP
            # produces a NEFF that fails at LoadExecutable.
            nc.sync.dma_start(
                out=out.rearrange("b n -> (b n)")[None, :], in_=out_sb[:]
            )

        if repeats > 1:
            # timing-only: hardware loop keeps the NEFF small at huge R
            with tc.For_i(0, repeats, 1):
                one_pass()
        else:
            one_pass()


def _build():
    global _compiled
    if _compiled is not None:
        return _compiled
    import concourse.mybir as mybir
    import concourse.tile as tile
    from concourse import bacc

    nc = bacc.Bacc("TRN2", target_bir_lowering=False, debug=False, num_devices=N_CORES)
    x = nc.dram_tensor("x", [B_LOC, T, N], mybir.dt.float32, kind="ExternalInput").ap()
    w = nc.dram_tensor("w", [P, C], mybir.dt.float32, kind="ExternalInput").ap()
    out = nc.dram_tensor("out", [B_LOC, N], mybir.dt.float32, kind="ExternalOutput").ap()

    with tile.TileContext(nc) as tc:
        _emit(tc, out, x, w)
    nc.compile()
    _compiled = nc
    return nc


def _w_host() -> np.ndarray:
    # w2d[p, c] = w[c*128 + p] — lhsT column layout for the PE.
    return np.ascontiguousarray(_gauss_weights().reshape(C, P).T)


def run_sharded(spike_trains: np.ndarray, trace: bool = False):
    """Run the SPMD kernel; returns (out [64,1024], BassKernelResults)."""
    from concourse.bass_utils import run_bass_kernel_spmd

    nc = _build()
    w2d = _w_host()
    x = np.ascontiguousarray(spike_trains, dtype=np.float32)
    in_maps = [
        {"x": x[i * B_LOC : (i + 1) * B_LOC], "w": w2d} for i in range(N_CORES)
    ]
    try:
        res = run_bass_kernel_spmd(nc, in_maps, list(range(N_CORES)), trace=trace)
    except Exception:
        # transient axon-terminal wedges (LoadExecutable/unrecoverable) heal
        # on retry; the NEFF is cached so this is cheap
        res = run_bass_kernel_spmd(nc, in_maps, list(range(N_CORES)), trace=trace)
    out = np.concatenate([res.results[i]["out"] for i in range(N_CORES)], axis=0)
    return out, res


def kernel(spike_trains: np.ndarray) -> np.ndarray:
    out, _ = run_sharded(spike_trains, trace=False)
    return out

